# revision 13
# baseline (speedup 1.0000x reference)
"""Trainium2 Bass kernel: single-head attention (B=4, S=2048, D=1024) on 8 NeuronCores.

Sharding: data-parallel over (batch, query-half): core c handles batch c//2,
query rows [c%2*1024, (c%2+1)*1024). No collectives.

Key algebraic fold: with zero q/k biases,
  scores = (Xq Wq)(Xkv Wk)^T = Xq (Wq Wk^T) Xkv^T = Xq M Xkv^T
with M = Wq Wk^T precomputed once on host (weights-only). This removes the
K-projection (2.1 GF/core) and the K^T AllGather of the previous design.
Per-core FLOPs: 12.88 GF -> 164 us PE floor at 78.6 TF/s bf16.

Nonzero bq/bk are handled by the augmented form (rare path, na=9 tiles):
  scores = [Xq 1] [[M, Wq bk],[(Wk bq)^T, bq.bk]] [Xkv 1]^T
zero-padded from 1025 to 1152 rows/cols. Mask and bv keep their own paths
(mask: pre-scaled add before exp; bv: out accumulates unnormalized and bias
enters as sums[q]*bv so the final 1/sums scale leaves exactly +bv).

Math per core (all matmuls bf16, fp32 PSUM accumulation):
  PT[k,q]   = M-blocks(lhsT) . XqT(rhs)           P = Xq M
  sT[s,q]   = XkvT-tiles(lhsT) . PT(rhs)          scores^T
  eT[s,q]   = exp(sT / sqrt(dk) [+ maskT])        ScalarE, PSUM->SBUF bf16
  sums      = DVE adds over eT s-tiles, then ones-matmul + reciprocal
  HT[v,q]   = Xkv-tiles(lhsT) . eT(rhs)           H = probs_unnorm @ Xkv
  out[q,dv] = HT-tiles(lhsT) . Wv(rhs)  (+bv)     out = H @ Wv
  out      *= 1/sums  (per-partition scale on ScalarE, fused with PSUM->SBUF)

PE group order (zero inter-stage stalls by construction: each stage's first
group depends only on work finished many groups earlier):
  A(c=0..2, i)  ->  B(n=0, m=0..15), B(n=1, m)  ->  D(n=0, j), D(n=1, j)
  ->  E(p=0..3), E(p=4..7); sums emitted inside E p=0.
Stage A runs chunk widths [256,256,512]: the first groups need only
m-block 0 + a 512KB lead of XqT, shrinking the startup-critical DMA set.

DMA design (from v1 trace analysis): aggregate startup bandwidth scales with
in-flight dma_starts; keep payloads ~128-512KB, fan out across sync, gpsimd
and vector queues (scalar gets only 2 issues so its first activation isn't
delayed; vector is free until the sacc chain at ~85us). All inputs are
host-packed into SBUF tile layouts so every dma_start is fully contiguous.
"""

import os
import numpy as np
import ml_dtypes

B, S, D = 4, 2048, 1024
N_CORES = 8
QL = S // 2  # query rows per core (1024)
NT_S = S // 128  # 16 s-tiles
BF16 = ml_dtypes.bfloat16

_cache: dict = {}

AQCH = [512, 512]  # stage-A chunk widths (sum = QL)
AOFF = [0, 512]


def _build(na: int, with_mask: bool, with_bv: bool, ps_bufs: int = 5):
    import concourse.bass as bass
    import concourse.mybir as mybir
    import concourse.tile as tile
    from concourse import bacc

    fp32 = mybir.dt.float32
    bf16 = mybir.dt.bfloat16

    nc = bacc.Bacc("TRN2", target_bir_lowering=False, debug=False,
                   num_devices=N_CORES)

    KA = na * 128          # augmented d_model/d_key (1024 or 1152)
    NQ = QL // 512         # 2 query chunks of 512
    NV = D // 128          # 8 v-blocks
    SCALE = 1.0 / float(np.sqrt(D))

    m_d = nc.dram_tensor("m", (128, na * KA), bf16, kind="ExternalInput")
    xq_d = nc.dram_tensor("xqt", (128, na * QL), bf16, kind="ExternalInput")
    kt_d = nc.dram_tensor("xkvt", (128, NT_S * KA), bf16, kind="ExternalInput")
    xkv_d = nc.dram_tensor("xkv", (128, NV * S), bf16, kind="ExternalInput")
    wv_d = nc.dram_tensor("wv", (128, NV * D), bf16, kind="ExternalInput")
    if with_bv:
        bv_d = nc.dram_tensor("bv", (1, D), bf16, kind="ExternalInput")
    if with_mask:
        maskt_d = nc.dram_tensor("maskt", (S, QL), bf16, kind="ExternalInput")
    out_d = nc.dram_tensor("out", (QL, D), bf16, kind="ExternalOutput")

    with tile.TileContext(nc) as tc:
        with (
            tc.tile_pool(name="cons", bufs=1) as cons,
            tc.tile_pool(name="mp", bufs=1) as mp,
            tc.tile_pool(name="xqp", bufs=1) as xqp,
            tc.tile_pool(name="ktp", bufs=1) as ktp,
            tc.tile_pool(name="ptp", bufs=1) as ptp,
            tc.tile_pool(name="etp", bufs=1) as etp,
            tc.tile_pool(name="xkvp", bufs=1) as xkvp,
            tc.tile_pool(name="wvp", bufs=1) as wvp,
            tc.tile_pool(name="htp", bufs=1) as htp,
            tc.tile_pool(name="outp", bufs=2) as outp,
            tc.tile_pool(name="maskp", bufs=2) as maskp,
            tc.tile_pool(name="ps", bufs=ps_bufs,
                         space=bass.MemorySpace.PSUM) as psp,
            tc.tile_pool(name="pss", bufs=1, space=bass.MemorySpace.PSUM) as pssp,
            tc.tile_pool(name="pst", bufs=1, space=bass.MemorySpace.PSUM) as pstp,
        ):
            sy, sc, gp, ve = nc.sync, nc.scalar, nc.gpsimd, nc.vector

            # ---- SBUF tiles ----
            m_blks = [mp.tile([128, KA], bf16, tag=f"m{i}", name=f"m{i}")
                      for i in range(na)]
            xq_chs = [xqp.tile([128, na, AQCH[c]], bf16, tag=f"xq{c}",
                               name=f"xq{c}") for c in range(len(AQCH))]
            kt_sb = ktp.tile([128, NT_S, KA], bf16, tag="kt")
            pt_sb = ptp.tile([128, na, QL], bf16, tag="pt")
            et_sb = etp.tile([128, NT_S, QL], bf16, tag="et")
            xkv_sb = xkvp.tile([128, NV, NT_S, 128], bf16, tag="xkv")
            wv_sb = wvp.tile([128, NV, D], bf16, tag="wv")
            ht_sb = htp.tile([128, NV, QL], bf16, tag="ht")

            # ---- DMA issue schedule (priority order per engine) ----
            def ld_m(eng, i, lo=0, hi=None):
                hi = KA if hi is None else hi
                eng.dma_start(m_blks[i][:, lo:hi],
                              m_d.ap()[:, i * KA + lo:i * KA + hi])

            def ld_xq(eng, c, jlo, jhi):
                w = AQCH[c]
                base = na * AOFF[c]
                eng.dma_start(xq_chs[c][:, jlo:jhi, :],
                              xq_d.ap()[:, base + jlo * w:base + jhi * w])

            def ld_kt(eng, mlo, mhi):
                eng.dma_start(kt_sb[:, mlo:mhi, :],
                              kt_d.ap()[:, mlo * KA:mhi * KA])

            def ld_xkv(eng, j):
                eng.dma_start(xkv_sb[:, j, :, :],
                              xkv_d.ap()[:, j * S:(j + 1) * S])

            def ld_wv(eng, jlo, jhi):
                eng.dma_start(wv_sb[:, jlo:jhi, :],
                              wv_d.ap()[:, jlo * D:jhi * D])

            # startup-critical wave: first A group needs m0 + xq chunk 0;
            # m[i] is then consumed every ~1.7us. sync/scalar are HWDGE
            # queues (first payload ~8.4us from cold); gpsimd is SWDGE
            # (~2us slower to first byte) so it only gets pieces needed
            # a few us into stage A. Tiny dedicated leads land first.
            sy.dma_start(m_blks[0][:, 0:128], m_d.ap()[:, 0:128])
            sy.dma_start(xq_chs[0][:, 0:1, :], xq_d.ap()[:, 0:AQCH[0]])
            sc.dma_start(m_blks[0][:, 128:512], m_d.ap()[:, 128:512])
            gp.dma_start(xq_chs[0][:, 4:na, :],
                         xq_d.ap()[:, 4 * AQCH[0]:na * AQCH[0]])
            sy.dma_start(m_blks[0][:, 512:KA], m_d.ap()[:, 512:KA])
            sc.dma_start(xq_chs[0][:, 1:2, :],
                         xq_d.ap()[:, AQCH[0]:2 * AQCH[0]])
            sy.dma_start(xq_chs[0][:, 2:4, :],
                         xq_d.ap()[:, 2 * AQCH[0]:4 * AQCH[0]])
            ld_m(sy, 1)
            ld_m(gp, 2)
            ld_m(sy, 3); ld_m(gp, 4)
            ld_m(sy, 5); ld_m(gp, 6)
            ld_m(sy, 7)
            if na > 8:
                ld_m(gp, 8)

            # stage-B lhsT: 16 s-blocks streamed during B_n0 (from ~25us,
            # one per 1.7us); reused by B_n1.
            for m4 in range(0, NT_S, 4):
                ld_kt(gp, m4, m4 + 2)
                ld_kt(sy, m4 + 2, m4 + 4)

            # xq chunk 1 feeds A_c1 (~52us)
            ld_xq(sy, 1, 0, 4); ld_xq(gp, 1, 4, na)

            # stage-D operands (from ~94us) and stage-E weights (~150us)
            for j in range(NV):
                ld_xkv((sy, gp)[j % 2], j)
            ld_wv(gp, 0, 2); ld_wv(sy, 2, 4)
            ld_wv(gp, 4, 6); ld_wv(sy, 6, 8)

            # ---- constants (tiny, off the critical path) ----
            ones_col = cons.tile([128, 1], fp32, tag="ones_col")
            gp.memset(ones_col[:], 1.0)
            ident1 = cons.tile([1, 1], fp32, tag="ident1")
            gp.memset(ident1[:], 1.0)

            # PE p-state priming: engines clear the NEFF preamble ~6.5us in
            # but the first operands only land ~9-10us. Dummy matmuls over a
            # DVE-memset tile keep the PE clock ramping through that window
            # so stage A starts near full speed.
            dummy_sb = cons.tile([128, 512], bf16, tag="dummy")
            ve.memset(dummy_sb[:], 0.0)
            for _ in range(8):
                psd = psp.tile([128, 512], fp32, tag="ps")
                nc.tensor.matmul(psd[:], dummy_sb[:, 0:128], dummy_sb[:],
                                 start=True, stop=True)
            if with_bv:
                bv_sb = cons.tile([1, D], bf16, tag="bv")
                sy.dma_start(bv_sb[:], bv_d.ap()[:])

            # ---- stages A and B, interleaved by q-chunk ----
            # A_c0, B_n0, A_c1, B_n1: spreads the startup-critical DMA set
            # (m + xq chunk 0, then kt streamed during B_n0) across ~50us of
            # compute instead of demanding all of m in stage-A's first 7us.
            def a_chunk(c):
                w, off = AQCH[c], AOFF[c]
                for i in range(na):
                    ps = psp.tile([128, 512], fp32, tag="ps")
                    for j in range(na):
                        nc.tensor.matmul(
                            ps[:, :w], m_blks[i][:, j * 128:(j + 1) * 128],
                            xq_chs[c][:, j, :],
                            start=(j == 0), stop=(j == na - 1))
                    nc.scalar.activation(
                        pt_sb[:, i, off:off + w], ps[:, :w],
                        mybir.ActivationFunctionType.Copy)

            def b_chunk(n):
                for m in range(NT_S):
                    ps = psp.tile([128, 512], fp32, tag="ps")
                    for j in range(na):
                        nc.tensor.matmul(
                            ps[:], kt_sb[:, m, j * 128:(j + 1) * 128],
                            pt_sb[:, j, n * 512:(n + 1) * 512],
                            start=(j == 0), stop=(j == na - 1))
                    if with_mask:
                        mk = maskp.tile([128, 512], bf16, tag="mask")
                        sy.dma_start(
                            mk[:], maskt_d.ap()[m * 128:(m + 1) * 128,
                                                n * 512:(n + 1) * 512])
                        nc.vector.tensor_tensor(
                            ps[:], ps[:], mk[:], mybir.AluOpType.add)
                    nc.scalar.activation(
                        et_sb[:, m, n * 512:(n + 1) * 512], ps[:],
                        mybir.ActivationFunctionType.Exp, scale=SCALE)

            a_chunk(0)
            b_chunk(0)
            a_chunk(1)
            b_chunk(1)

            # softmax denominators on the DVE (PE has no slack; DVE does).
            sacc = cons.tile([128, QL], fp32, tag="sacc")
            nc.vector.tensor_tensor(
                sacc[:], et_sb[:, 0, :], et_sb[:, 1, :], mybir.AluOpType.add)
            for m in range(2, NT_S):
                nc.vector.tensor_tensor(
                    sacc[:], sacc[:], et_sb[:, m, :], mybir.AluOpType.add)

            sums_sb = cons.tile([1, QL], fp32, tag="sums")
            pst = pstp.tile([128, 8], fp32, tag="pst")
            recip_sb = cons.tile([128, 8], fp32, tag="recip")
            if with_bv:
                sums_bf = cons.tile([1, QL], bf16, tag="sums_bf")

            # sums pieces woven between stage-D groups: the pss bank (bufs=1)
            # serializes on its ScalarE drain, so a 3.4us D group between the
            # two sums matmuls hides every handoff.
            def sums_piece(k):
                if k < NQ:
                    pss = pssp.tile([1, 512], fp32, tag="pss")
                    nc.tensor.matmul(
                        pss[:], ones_col[:], sacc[:, k * 512:(k + 1) * 512],
                        start=True, stop=True)
                    nc.scalar.activation(
                        sums_sb[:, k * 512:(k + 1) * 512], pss[:],
                        mybir.ActivationFunctionType.Copy)
                elif k < 4:
                    for p in range(4 * (k - 2), 4 * (k - 1)):
                        nc.tensor.transpose(
                            pst[:, p:p + 1], sums_sb[:, p * 128:(p + 1) * 128],
                            ident1[:])
                elif k == 4:
                    nc.vector.reciprocal(recip_sb[:], pst[:])
                    if with_bv:
                        # out accumulates UNNORMALIZED; bias enters as
                        # sums[q]*bv so the 1/sums scale leaves exactly +bv
                        nc.scalar.activation(
                            sums_bf[:], sums_sb[:],
                            mybir.ActivationFunctionType.Copy)

            # ---- stage D: HT = Xkv-tiles . eT ----
            for n in range(NQ):
                for j in range(NV):
                    ps = psp.tile([128, 512], fp32, tag="ps")
                    for m in range(NT_S):
                        nc.tensor.matmul(
                            ps[:], xkv_sb[:, j, m, :],
                            et_sb[:, m, n * 512:(n + 1) * 512],
                            start=(m == 0), stop=(m == NT_S - 1))
                    nc.scalar.activation(
                        ht_sb[:, j, n * 512:(n + 1) * 512], ps[:],
                        mybir.ActivationFunctionType.Copy)
                    if n == 1 and j >= 1:
                        sums_piece(j - 1)

            # ---- stage E: out = HT^T . Wv (+bv), normalized, bf16 out ----
            # p=0: matmuls first, then the sums block, then normalization.
            # The last p runs its final 512 cols as two 256-col groups with
            # act+store flushed per group, so only one short act+DMA chain
            # trails the last matmul.
            for p in range(8):
                out_sb = outp.tile([128, D], bf16, tag="outsb")
                widths = ([(0, 512), (512, 512)] if p < 7
                          else [(0, 512), (512, 256), (768, 256)])
                pending = []

                def flush(p=p, out_sb=out_sb, pending=pending):
                    for ps, lo, w in pending:
                        if with_bv:
                            nc.tensor.matmul(
                                ps[:, :w], sums_bf[:, p * 128:(p + 1) * 128],
                                bv_sb[:, lo:lo + w],
                                start=False, stop=True)
                        nc.scalar.activation(
                            out_sb[:, lo:lo + w], ps[:, :w],
                            mybir.ActivationFunctionType.Copy,
                            scale=recip_sb[:, p:p + 1])
                        # p=7 stores all go on sync: a store issued on the
                        # scalar stream would delay the final activations
                        eng = (nc.sync if p == 7 or (p + lo // 512) % 2 == 0
                               else nc.scalar)
                        eng.dma_start(
                            out_d.ap()[p * 128:(p + 1) * 128, lo:lo + w],
                            out_sb[:, lo:lo + w])
                    pending.clear()

                for lo, w in widths:
                    ps = psp.tile([128, 512], fp32, tag="ps")
                    for j in range(NV):
                        nc.tensor.matmul(
                            ps[:, :w], ht_sb[:, j, p * 128:(p + 1) * 128],
                            wv_sb[:, j, lo:lo + w],
                            start=(j == 0),
                            stop=(j == NV - 1 and not with_bv))
                    pending.append((ps, lo, w))
                    flush()

    nc.compile()
    return nc


def _get_nc(flags):
    if flags not in _cache:
        _cache[flags] = _build(*flags)
    return _cache[flags]


def _pack_blocks(X, nj, ni):
    # X [nj*128, ni*128] -> [128, ni*nj*128]: out[p, (i*nj+j)*128+c]
    # = X[j*128+p, i*128+c] (i-block-major SBUF tile layout, each i-block
    # one fully-contiguous DMA)
    return np.ascontiguousarray(
        X.reshape(nj, 128, ni, 128).transpose(1, 2, 0, 3)
        .reshape(128, ni * nj * 128))


def _pack_x(xt, widths, na):
    # xt [na*128, L] -> [128, sum_c na*w_c]: per col-chunk c,
    # block[p, j*w + t] = xt[j*128+p, off+t]
    blocks = []
    off = 0
    for w in widths:
        blk = xt[:, off:off + w].reshape(na, 128, w).transpose(1, 0, 2)
        blocks.append(blk.reshape(128, na * w))
        off += w
    return np.ascontiguousarray(np.concatenate(blocks, axis=1))


def _prep_in_maps(query_input, keyvalue_input, mask, Wq, bq, Wk, bk, Wv, bv):
    qi = np.asarray(query_input, np.float32)
    kv = np.asarray(keyvalue_input, np.float32)
    mask = np.asarray(mask, np.float32)
    Wq = np.asarray(Wq, np.float32)
    Wk = np.asarray(Wk, np.float32)
    Wvb = np.asarray(Wv, np.float32).astype(BF16)
    bq = np.asarray(bq, np.float32)
    bk = np.asarray(bk, np.float32)
    bv = np.asarray(bv, np.float32)

    with_mask = bool(np.any(mask != 0.0))
    with_bias = bool(np.any(bq != 0.0) or np.any(bk != 0.0))
    with_bv = bool(np.any(bv != 0.0))
    na = 9 if with_bias else 8
    KA = na * 128
    flags = (na, with_mask, with_bv)

    # M = Wq Wk^T (weights-only fold), augmented with q/k biases if nonzero
    M = np.zeros((KA, KA), np.float32)
    M[:D, :D] = Wq @ Wk.T
    if with_bias:
        M[:D, D] = Wq @ bk
        M[D, :D] = Wk @ bq
        M[D, D] = float(bq @ bk)
    M_p = _pack_blocks(M.astype(BF16), na, na)
    wv_p = np.ascontiguousarray(
        Wvb.reshape(8, 128, D).transpose(1, 0, 2).reshape(128, 8 * D))

    in_maps = []
    for c in range(N_CORES):
        b, h = c // 2, c % 2
        xq = qi[b, h * QL:(h + 1) * QL, :]          # [QL, D] fp32
        xkv = kv[b]                                  # [S, D] fp32
        xqt = np.zeros((KA, QL), np.float32)
        xqt[:D] = xq.T
        xkvt = np.zeros((KA, S), np.float32)
        xkvt[:D] = xkv.T
        if with_bias:
            xqt[D] = 1.0
            xkvt[D] = 1.0
        m_ = {
            "m": M_p,
            "xqt": _pack_x(xqt.astype(BF16), AQCH, na),
            "xkvt": _pack_blocks(xkvt.astype(BF16), na, NT_S),
            "xkv": _pack_blocks(xkv.astype(BF16), NT_S, 8),
            "wv": wv_p,
        }
        if with_bv:
            m_["bv"] = bv.astype(BF16).reshape(1, D)
        if with_mask:
            mt = mask[b, h * QL:(h + 1) * QL, :].T * np.float32(np.sqrt(D))
            m_["maskt"] = np.ascontiguousarray(mt.astype(BF16))
        in_maps.append(m_)
    return flags, in_maps


def _ensure_axon_hooks_stub():
    # bass_utils imports antenv.axon_hooks when tracing is requested; the
    # module is absent on some images, so register a no-op stub if needed.
    import sys, types
    try:
        import antenv.axon_hooks  # noqa: F401
    except ImportError:
        stub = types.ModuleType("antenv.axon_hooks")
        stub._hook = None
        stub.set_axon_ntff_profile_hook = (
            lambda h: setattr(stub, "_hook", h))
        stub.get_axon_ntff_profile_hook = lambda: stub._hook
        sys.modules["antenv.axon_hooks"] = stub
        try:
            import antenv
            antenv.axon_hooks = stub
        except ImportError:
            pass


def _run(inputs, trace=False, **kw):
    _ensure_axon_hooks_stub()
    from concourse import bass_utils
    ps_bufs = int(os.environ.get("KERNEL_PSBUFS", "6"))
    flags, in_maps = _prep_in_maps(**inputs)
    nc = _get_nc(flags + (ps_bufs,))
    res = bass_utils.run_bass_kernel_spmd(
        nc, in_maps, core_ids=list(range(N_CORES)), trace=trace, **kw)
    out = np.empty((B, S, D), np.float32)
    for c in range(N_CORES):
        b, h = c // 2, c % 2
        out[b, h * QL:(h + 1) * QL, :] = np.asarray(
            res.results[c]["out"], dtype=np.float32)
    return out, res


def kernel(**inputs) -> np.ndarray:
    out, _ = _run(inputs, trace=False)
    return out


# revision 15
# speedup vs baseline: 1.0041x; 1.0041x over previous
"""Trainium2 Bass kernel: single-head attention (B=4, S=2048, D=1024) on 8 NeuronCores.

Sharding: data-parallel over (batch, query-half): core c handles batch c//2,
query rows [c%2*1024, (c%2+1)*1024). No collectives.

Key algebraic fold: with zero q/k biases,
  scores = (Xq Wq)(Xkv Wk)^T = Xq (Wq Wk^T) Xkv^T = Xq M Xkv^T
with M = Wq Wk^T precomputed once on host (weights-only). This removes the
K-projection (2.1 GF/core) and the K^T AllGather of the previous design.
Per-core FLOPs: 12.88 GF -> 164 us PE floor at 78.6 TF/s bf16.

Nonzero bq/bk are handled by the augmented form (rare path, na=9 tiles):
  scores = [Xq 1] [[M, Wq bk],[(Wk bq)^T, bq.bk]] [Xkv 1]^T
zero-padded from 1025 to 1152 rows/cols. Mask and bv keep their own paths
(mask: pre-scaled add before exp; bv: out accumulates unnormalized and bias
enters as sums[q]*bv so the final 1/sums scale leaves exactly +bv).

Math per core (all matmuls bf16, fp32 PSUM accumulation):
  PT[k,q]   = M-blocks(lhsT) . XqT(rhs)           P = Xq M
  sT[s,q]   = XkvT-tiles(lhsT) . PT(rhs)          scores^T
  eT[s,q]   = exp(sT / sqrt(dk) [+ maskT])        ScalarE, PSUM->SBUF bf16
  sums      = DVE adds over eT s-tiles, then ones-matmul + reciprocal
  HT[v,q]   = Xkv-tiles(lhsT) . eT(rhs)           H = probs_unnorm @ Xkv
  out[q,dv] = HT-tiles(lhsT) . Wv(rhs)  (+bv)     out = H @ Wv
  out      *= 1/sums  (per-partition scale on ScalarE, fused with PSUM->SBUF)

PE group order (zero inter-stage stalls by construction: each stage's first
group depends only on work finished many groups earlier):
  A(c=0..2, i)  ->  B(n=0, m=0..15), B(n=1, m)  ->  D(n=0, j), D(n=1, j)
  ->  E(p=0..3), E(p=4..7); sums emitted inside E p=0.
Stage A runs chunk widths [256,256,512]: the first groups need only
m-block 0 + a 512KB lead of XqT, shrinking the startup-critical DMA set.

DMA design (from v1 trace analysis): aggregate startup bandwidth scales with
in-flight dma_starts; keep payloads ~128-512KB, fan out across sync, gpsimd
and vector queues (scalar gets only 2 issues so its first activation isn't
delayed; vector is free until the sacc chain at ~85us). All inputs are
host-packed into SBUF tile layouts so every dma_start is fully contiguous.
"""

import os
import numpy as np
import ml_dtypes

B, S, D = 4, 2048, 1024
N_CORES = 8
QL = S // 2  # query rows per core (1024)
NT_S = S // 128  # 16 s-tiles
BF16 = ml_dtypes.bfloat16

_cache: dict = {}

AQCH = [512, 512]  # stage-A chunk widths (sum = QL)
AOFF = [0, 512]


def _build(na: int, with_mask: bool, with_bv: bool, ps_bufs: int = 5):
    import concourse.bass as bass
    import concourse.mybir as mybir
    import concourse.tile as tile
    from concourse import bacc

    fp32 = mybir.dt.float32
    bf16 = mybir.dt.bfloat16

    nc = bacc.Bacc("TRN2", target_bir_lowering=False, debug=False,
                   num_devices=N_CORES)

    KA = na * 128          # augmented d_model/d_key (1024 or 1152)
    NQ = QL // 512         # 2 query chunks of 512
    NV = D // 128          # 8 v-blocks
    SCALE = 1.0 / float(np.sqrt(D))

    m_d = nc.dram_tensor("m", (128, na * KA), bf16, kind="ExternalInput")
    xq_d = nc.dram_tensor("xqt", (128, na * QL), bf16, kind="ExternalInput")
    kt_d = nc.dram_tensor("xkvt", (128, NT_S * KA), bf16, kind="ExternalInput")
    xkv_d = nc.dram_tensor("xkv", (128, NV * S), bf16, kind="ExternalInput")
    wv_d = nc.dram_tensor("wv", (128, NV * D), bf16, kind="ExternalInput")
    if with_bv:
        bv_d = nc.dram_tensor("bv", (1, D), bf16, kind="ExternalInput")
    if with_mask:
        maskt_d = nc.dram_tensor("maskt", (S, QL), bf16, kind="ExternalInput")
    out_d = nc.dram_tensor("out", (QL, D), bf16, kind="ExternalOutput")

    with tile.TileContext(nc) as tc:
        with (
            tc.tile_pool(name="cons", bufs=1) as cons,
            tc.tile_pool(name="mp", bufs=1) as mp,
            tc.tile_pool(name="xqp", bufs=1) as xqp,
            tc.tile_pool(name="ktp", bufs=1) as ktp,
            tc.tile_pool(name="ptp", bufs=1) as ptp,
            tc.tile_pool(name="etp", bufs=1) as etp,
            tc.tile_pool(name="xkvp", bufs=1) as xkvp,
            tc.tile_pool(name="wvp", bufs=1) as wvp,
            tc.tile_pool(name="htp", bufs=1) as htp,
            tc.tile_pool(name="outp", bufs=2) as outp,
            tc.tile_pool(name="maskp", bufs=2) as maskp,
            tc.tile_pool(name="ps", bufs=ps_bufs,
                         space=bass.MemorySpace.PSUM) as psp,
            tc.tile_pool(name="pss", bufs=1, space=bass.MemorySpace.PSUM) as pssp,
            tc.tile_pool(name="pst", bufs=1, space=bass.MemorySpace.PSUM) as pstp,
        ):
            sy, sc, gp, ve = nc.sync, nc.scalar, nc.gpsimd, nc.vector

            # ---- SBUF tiles ----
            m_blks = [mp.tile([128, KA], bf16, tag=f"m{i}", name=f"m{i}")
                      for i in range(na)]
            xq_chs = [xqp.tile([128, na, AQCH[c]], bf16, tag=f"xq{c}",
                               name=f"xq{c}") for c in range(len(AQCH))]
            kt_sb = ktp.tile([128, NT_S, KA], bf16, tag="kt")
            pt_sb = ptp.tile([128, na, QL], bf16, tag="pt")
            et_sb = etp.tile([128, NT_S, QL], bf16, tag="et")
            xkv_sb = xkvp.tile([128, NV, NT_S, 128], bf16, tag="xkv")
            wv_sb = wvp.tile([128, NV, D], bf16, tag="wv")
            ht_sb = htp.tile([128, NV, QL], bf16, tag="ht")

            # ---- DMA issue schedule (priority order per engine) ----
            def ld_m(eng, i, lo=0, hi=None):
                hi = KA if hi is None else hi
                eng.dma_start(m_blks[i][:, lo:hi],
                              m_d.ap()[:, i * KA + lo:i * KA + hi])

            def ld_xq(eng, c, jlo, jhi):
                w = AQCH[c]
                base = na * AOFF[c]
                eng.dma_start(xq_chs[c][:, jlo:jhi, :],
                              xq_d.ap()[:, base + jlo * w:base + jhi * w])

            def ld_kt(eng, mlo, mhi):
                eng.dma_start(kt_sb[:, mlo:mhi, :],
                              kt_d.ap()[:, mlo * KA:mhi * KA])

            def ld_xkv(eng, j):
                eng.dma_start(xkv_sb[:, j, :, :],
                              xkv_d.ap()[:, j * S:(j + 1) * S])

            def ld_wv(eng, jlo, jhi):
                eng.dma_start(wv_sb[:, jlo:jhi, :],
                              wv_d.ap()[:, jlo * D:jhi * D])

            # startup-critical wave: first A group needs m0 + xq chunk 0;
            # m[i] is then consumed every ~1.7us. sync/scalar are HWDGE
            # queues (first payload ~8.4us from cold); gpsimd is SWDGE
            # (~2us slower to first byte) so it only gets pieces needed
            # a few us into stage A. Tiny dedicated leads land first.
            sy.dma_start(m_blks[0][:, 0:128], m_d.ap()[:, 0:128])
            sy.dma_start(xq_chs[0][:, 0:1, :], xq_d.ap()[:, 0:AQCH[0]])
            sc.dma_start(m_blks[0][:, 128:512], m_d.ap()[:, 128:512])
            gp.dma_start(xq_chs[0][:, 4:na, :],
                         xq_d.ap()[:, 4 * AQCH[0]:na * AQCH[0]])
            sy.dma_start(xq_chs[0][:, 2:4, :],
                         xq_d.ap()[:, 2 * AQCH[0]:4 * AQCH[0]])
            sc.dma_start(xq_chs[0][:, 1:2, :],
                         xq_d.ap()[:, AQCH[0]:2 * AQCH[0]])
            sy.dma_start(m_blks[0][:, 512:KA], m_d.ap()[:, 512:KA])
            ld_m(sy, 1)
            ld_m(gp, 2)
            ld_m(sy, 3); ld_m(gp, 4)
            ld_m(sy, 5); ld_m(gp, 6)
            ld_m(sy, 7)
            if na > 8:
                ld_m(gp, 8)

            # stage-B lhsT: 16 s-blocks streamed during B_n0 (from ~25us,
            # one per 1.7us); reused by B_n1.
            for m4 in range(0, NT_S, 4):
                ld_kt(gp, m4, m4 + 2)
                ld_kt(sy, m4 + 2, m4 + 4)

            # xq chunk 1 feeds A_c1 (~52us)
            ld_xq(sy, 1, 0, 4); ld_xq(gp, 1, 4, na)

            # stage-D operands (from ~94us) and stage-E weights (~150us)
            for j in range(NV):
                ld_xkv((sy, gp)[j % 2], j)
            ld_wv(gp, 0, 2); ld_wv(sy, 2, 4)
            ld_wv(gp, 4, 6); ld_wv(sy, 6, 8)

            # ---- constants (tiny, off the critical path) ----
            ones_col = cons.tile([128, 1], fp32, tag="ones_col")
            gp.memset(ones_col[:], 1.0)
            ident1 = cons.tile([1, 1], fp32, tag="ident1")
            gp.memset(ident1[:], 1.0)


            if with_bv:
                bv_sb = cons.tile([1, D], bf16, tag="bv")
                sy.dma_start(bv_sb[:], bv_d.ap()[:])

            # ---- stages A and B, interleaved by q-chunk ----
            # A_c0, B_n0, A_c1, B_n1: spreads the startup-critical DMA set
            # (m + xq chunk 0, then kt streamed during B_n0) across ~50us of
            # compute instead of demanding all of m in stage-A's first 7us.
            def a_chunk(c):
                w, off = AQCH[c], AOFF[c]
                for i in range(na):
                    ps = psp.tile([128, 512], fp32, tag="ps")
                    for j in range(na):
                        nc.tensor.matmul(
                            ps[:, :w], m_blks[i][:, j * 128:(j + 1) * 128],
                            xq_chs[c][:, j, :],
                            start=(j == 0), stop=(j == na - 1))
                    nc.scalar.activation(
                        pt_sb[:, i, off:off + w], ps[:, :w],
                        mybir.ActivationFunctionType.Copy)

            def b_chunk(n):
                for m in range(NT_S):
                    ps = psp.tile([128, 512], fp32, tag="ps")
                    for j in range(na):
                        nc.tensor.matmul(
                            ps[:], kt_sb[:, m, j * 128:(j + 1) * 128],
                            pt_sb[:, j, n * 512:(n + 1) * 512],
                            start=(j == 0), stop=(j == na - 1))
                    if with_mask:
                        mk = maskp.tile([128, 512], bf16, tag="mask")
                        sy.dma_start(
                            mk[:], maskt_d.ap()[m * 128:(m + 1) * 128,
                                                n * 512:(n + 1) * 512])
                        nc.vector.tensor_tensor(
                            ps[:], ps[:], mk[:], mybir.AluOpType.add)
                    nc.scalar.activation(
                        et_sb[:, m, n * 512:(n + 1) * 512], ps[:],
                        mybir.ActivationFunctionType.Exp, scale=SCALE)

            a_chunk(0)
            b_chunk(0)
            a_chunk(1)
            b_chunk(1)

            # softmax denominators on the DVE (PE has no slack; DVE does).
            sacc = cons.tile([128, QL], fp32, tag="sacc")
            nc.vector.tensor_tensor(
                sacc[:], et_sb[:, 0, :], et_sb[:, 1, :], mybir.AluOpType.add)
            for m in range(2, NT_S):
                nc.vector.tensor_tensor(
                    sacc[:], sacc[:], et_sb[:, m, :], mybir.AluOpType.add)

            sums_sb = cons.tile([1, QL], fp32, tag="sums")
            pst = pstp.tile([128, 8], fp32, tag="pst")
            recip_sb = cons.tile([128, 8], fp32, tag="recip")
            if with_bv:
                sums_bf = cons.tile([1, QL], bf16, tag="sums_bf")

            # sums pieces woven between stage-D groups: the pss bank (bufs=1)
            # serializes on its ScalarE drain, so a 3.4us D group between the
            # two sums matmuls hides every handoff.
            def sums_piece(k):
                if k < NQ:
                    pss = pssp.tile([1, 512], fp32, tag="pss")
                    nc.tensor.matmul(
                        pss[:], ones_col[:], sacc[:, k * 512:(k + 1) * 512],
                        start=True, stop=True)
                    nc.scalar.activation(
                        sums_sb[:, k * 512:(k + 1) * 512], pss[:],
                        mybir.ActivationFunctionType.Copy)
                elif k < 4:
                    for p in range(4 * (k - 2), 4 * (k - 1)):
                        nc.tensor.transpose(
                            pst[:, p:p + 1], sums_sb[:, p * 128:(p + 1) * 128],
                            ident1[:])
                elif k == 4:
                    nc.vector.reciprocal(recip_sb[:], pst[:])
                    if with_bv:
                        # out accumulates UNNORMALIZED; bias enters as
                        # sums[q]*bv so the 1/sums scale leaves exactly +bv
                        nc.scalar.activation(
                            sums_bf[:], sums_sb[:],
                            mybir.ActivationFunctionType.Copy)

            # ---- stage D: HT = Xkv-tiles . eT ----
            for n in range(NQ):
                for j in range(NV):
                    ps = psp.tile([128, 512], fp32, tag="ps")
                    for m in range(NT_S):
                        nc.tensor.matmul(
                            ps[:], xkv_sb[:, j, m, :],
                            et_sb[:, m, n * 512:(n + 1) * 512],
                            start=(m == 0), stop=(m == NT_S - 1))
                    nc.scalar.activation(
                        ht_sb[:, j, n * 512:(n + 1) * 512], ps[:],
                        mybir.ActivationFunctionType.Copy)
                    if n == 1 and j >= 1:
                        sums_piece(j - 1)

            # ---- stage E: out = HT^T . Wv (+bv), normalized, bf16 out ----
            # p=0: matmuls first, then the sums block, then normalization.
            # The last p runs its final 512 cols as two 256-col groups with
            # act+store flushed per group, so only one short act+DMA chain
            # trails the last matmul.
            for p in range(8):
                out_sb = outp.tile([128, D], bf16, tag="outsb")
                widths = ([(0, 512), (512, 512)] if p < 7
                          else [(0, 512), (512, 256), (768, 256)])
                pending = []

                def flush(p=p, out_sb=out_sb, pending=pending):
                    for ps, lo, w in pending:
                        if with_bv:
                            nc.tensor.matmul(
                                ps[:, :w], sums_bf[:, p * 128:(p + 1) * 128],
                                bv_sb[:, lo:lo + w],
                                start=False, stop=True)
                        nc.scalar.activation(
                            out_sb[:, lo:lo + w], ps[:, :w],
                            mybir.ActivationFunctionType.Copy,
                            scale=recip_sb[:, p:p + 1])
                        # p=7 stores all go on sync: a store issued on the
                        # scalar stream would delay the final activations
                        eng = (nc.sync if p == 7 or (p + lo // 512) % 2 == 0
                               else nc.scalar)
                        eng.dma_start(
                            out_d.ap()[p * 128:(p + 1) * 128, lo:lo + w],
                            out_sb[:, lo:lo + w])
                    pending.clear()

                for lo, w in widths:
                    ps = psp.tile([128, 512], fp32, tag="ps")
                    for j in range(NV):
                        nc.tensor.matmul(
                            ps[:, :w], ht_sb[:, j, p * 128:(p + 1) * 128],
                            wv_sb[:, j, lo:lo + w],
                            start=(j == 0),
                            stop=(j == NV - 1 and not with_bv))
                    pending.append((ps, lo, w))
                    flush()

    nc.compile()
    return nc


def _get_nc(flags):
    if flags not in _cache:
        _cache[flags] = _build(*flags)
    return _cache[flags]


def _pack_blocks(X, nj, ni):
    # X [nj*128, ni*128] -> [128, ni*nj*128]: out[p, (i*nj+j)*128+c]
    # = X[j*128+p, i*128+c] (i-block-major SBUF tile layout, each i-block
    # one fully-contiguous DMA)
    return np.ascontiguousarray(
        X.reshape(nj, 128, ni, 128).transpose(1, 2, 0, 3)
        .reshape(128, ni * nj * 128))


def _pack_x(xt, widths, na):
    # xt [na*128, L] -> [128, sum_c na*w_c]: per col-chunk c,
    # block[p, j*w + t] = xt[j*128+p, off+t]
    blocks = []
    off = 0
    for w in widths:
        blk = xt[:, off:off + w].reshape(na, 128, w).transpose(1, 0, 2)
        blocks.append(blk.reshape(128, na * w))
        off += w
    return np.ascontiguousarray(np.concatenate(blocks, axis=1))


def _prep_in_maps(query_input, keyvalue_input, mask, Wq, bq, Wk, bk, Wv, bv):
    qi = np.asarray(query_input, np.float32)
    kv = np.asarray(keyvalue_input, np.float32)
    mask = np.asarray(mask, np.float32)
    Wq = np.asarray(Wq, np.float32)
    Wk = np.asarray(Wk, np.float32)
    Wvb = np.asarray(Wv, np.float32).astype(BF16)
    bq = np.asarray(bq, np.float32)
    bk = np.asarray(bk, np.float32)
    bv = np.asarray(bv, np.float32)

    with_mask = bool(np.any(mask != 0.0))
    with_bias = bool(np.any(bq != 0.0) or np.any(bk != 0.0))
    with_bv = bool(np.any(bv != 0.0))
    na = 9 if with_bias else 8
    KA = na * 128
    flags = (na, with_mask, with_bv)

    # M = Wq Wk^T (weights-only fold), augmented with q/k biases if nonzero
    M = np.zeros((KA, KA), np.float32)
    M[:D, :D] = Wq @ Wk.T
    if with_bias:
        M[:D, D] = Wq @ bk
        M[D, :D] = Wk @ bq
        M[D, D] = float(bq @ bk)
    M_p = _pack_blocks(M.astype(BF16), na, na)
    wv_p = np.ascontiguousarray(
        Wvb.reshape(8, 128, D).transpose(1, 0, 2).reshape(128, 8 * D))

    in_maps = []
    for c in range(N_CORES):
        b, h = c // 2, c % 2
        xq = qi[b, h * QL:(h + 1) * QL, :]          # [QL, D] fp32
        xkv = kv[b]                                  # [S, D] fp32
        xqt = np.zeros((KA, QL), np.float32)
        xqt[:D] = xq.T
        xkvt = np.zeros((KA, S), np.float32)
        xkvt[:D] = xkv.T
        if with_bias:
            xqt[D] = 1.0
            xkvt[D] = 1.0
        m_ = {
            "m": M_p,
            "xqt": _pack_x(xqt.astype(BF16), AQCH, na),
            "xkvt": _pack_blocks(xkvt.astype(BF16), na, NT_S),
            "xkv": _pack_blocks(xkv.astype(BF16), NT_S, 8),
            "wv": wv_p,
        }
        if with_bv:
            m_["bv"] = bv.astype(BF16).reshape(1, D)
        if with_mask:
            mt = mask[b, h * QL:(h + 1) * QL, :].T * np.float32(np.sqrt(D))
            m_["maskt"] = np.ascontiguousarray(mt.astype(BF16))
        in_maps.append(m_)
    return flags, in_maps


def _ensure_axon_hooks_stub():
    # bass_utils imports antenv.axon_hooks when tracing is requested; the
    # module is absent on some images, so register a no-op stub if needed.
    import sys, types
    try:
        import antenv.axon_hooks  # noqa: F401
    except ImportError:
        stub = types.ModuleType("antenv.axon_hooks")
        stub._hook = None
        stub.set_axon_ntff_profile_hook = (
            lambda h: setattr(stub, "_hook", h))
        stub.get_axon_ntff_profile_hook = lambda: stub._hook
        sys.modules["antenv.axon_hooks"] = stub
        try:
            import antenv
            antenv.axon_hooks = stub
        except ImportError:
            pass


def _run(inputs, trace=False, **kw):
    _ensure_axon_hooks_stub()
    from concourse import bass_utils
    ps_bufs = int(os.environ.get("KERNEL_PSBUFS", "6"))
    flags, in_maps = _prep_in_maps(**inputs)
    nc = _get_nc(flags + (ps_bufs,))
    res = bass_utils.run_bass_kernel_spmd(
        nc, in_maps, core_ids=list(range(N_CORES)), trace=trace, **kw)
    out = np.empty((B, S, D), np.float32)
    for c in range(N_CORES):
        b, h = c // 2, c % 2
        out[b, h * QL:(h + 1) * QL, :] = np.asarray(
            res.results[c]["out"], dtype=np.float32)
    return out, res


def kernel(**inputs) -> np.ndarray:
    out, _ = _run(inputs, trace=False)
    return out


# revision 19
# speedup vs baseline: 1.0074x; 1.0034x over previous
"""Trainium2 Bass kernel: single-head attention (B=4, S=2048, D=1024) on 8 NeuronCores.

Sharding: data-parallel over (batch, query-half): core c handles batch c//2,
query rows [c%2*1024, (c%2+1)*1024). No collectives.

Key algebraic fold: with zero q/k biases,
  scores = (Xq Wq)(Xkv Wk)^T = Xq (Wq Wk^T) Xkv^T = Xq M Xkv^T
with M = Wq Wk^T precomputed once on host (weights-only). This removes the
K-projection (2.1 GF/core) and the K^T AllGather of the previous design.
Per-core FLOPs: 12.88 GF -> 164 us PE floor at 78.6 TF/s bf16.

Nonzero bq/bk are handled by the augmented form (rare path, na=9 tiles):
  scores = [Xq 1] [[M, Wq bk],[(Wk bq)^T, bq.bk]] [Xkv 1]^T
zero-padded from 1025 to 1152 rows/cols. Mask and bv keep their own paths
(mask: pre-scaled add before exp; bv: out accumulates unnormalized and bias
enters as sums[q]*bv so the final 1/sums scale leaves exactly +bv).

Math per core (all matmuls bf16, fp32 PSUM accumulation):
  PT[k,q]   = M-blocks(lhsT) . XqT(rhs)           P = Xq M
  sT[s,q]   = XkvT-tiles(lhsT) . PT(rhs)          scores^T
  eT[s,q]   = exp(sT / sqrt(dk) [+ maskT])        ScalarE, PSUM->SBUF bf16
  sums      = DVE adds over eT s-tiles, then ones-matmul + reciprocal
  HT[v,q]   = Xkv-tiles(lhsT) . eT(rhs)           H = probs_unnorm @ Xkv
  out[q,dv] = HT-tiles(lhsT) . Wv(rhs)  (+bv)     out = H @ Wv
  out      *= 1/sums  (per-partition scale on ScalarE, fused with PSUM->SBUF)

PE group order (zero inter-stage stalls by construction: each stage's first
group depends only on work finished many groups earlier):
  A(c=0..2, i)  ->  B(n=0, m=0..15), B(n=1, m)  ->  D(n=0, j), D(n=1, j)
  ->  E(p=0..3), E(p=4..7); sums emitted inside E p=0.
Stage A runs chunk widths [256,256,512]: the first groups need only
m-block 0 + a 512KB lead of XqT, shrinking the startup-critical DMA set.

DMA design (from v1 trace analysis): aggregate startup bandwidth scales with
in-flight dma_starts; keep payloads ~128-512KB, fan out across sync, gpsimd
and vector queues (scalar gets only 2 issues so its first activation isn't
delayed; vector is free until the sacc chain at ~85us). All inputs are
host-packed into SBUF tile layouts so every dma_start is fully contiguous.
"""

import os
import numpy as np
import ml_dtypes

B, S, D = 4, 2048, 1024
N_CORES = 8
QL = S // 2  # query rows per core (1024)
NT_S = S // 128  # 16 s-tiles
BF16 = ml_dtypes.bfloat16

_cache: dict = {}

AQCH = [512, 512]  # stage-A chunk widths (sum = QL)
AOFF = [0, 512]


def _build(na: int, with_mask: bool, with_bv: bool, ps_bufs: int = 5):
    import concourse.bass as bass
    import concourse.mybir as mybir
    import concourse.tile as tile
    from concourse import bacc

    fp32 = mybir.dt.float32
    bf16 = mybir.dt.bfloat16

    nc = bacc.Bacc("TRN2", target_bir_lowering=False, debug=False,
                   num_devices=N_CORES)

    KA = na * 128          # augmented d_model/d_key (1024 or 1152)
    NQ = QL // 512         # 2 query chunks of 512
    NV = D // 128          # 8 v-blocks
    SCALE = 1.0 / float(np.sqrt(D))

    m_d = nc.dram_tensor("m", (128, na * KA), bf16, kind="ExternalInput")
    xq_d = nc.dram_tensor("xqt", (128, na * QL), bf16, kind="ExternalInput")
    kt_d = nc.dram_tensor("xkvt", (128, NT_S * KA), bf16, kind="ExternalInput")
    xkv_d = nc.dram_tensor("xkv", (128, NV * S), bf16, kind="ExternalInput")
    wv_d = nc.dram_tensor("wv", (128, NV * D), bf16, kind="ExternalInput")
    if with_bv:
        bv_d = nc.dram_tensor("bv", (1, D), bf16, kind="ExternalInput")
    if with_mask:
        maskt_d = nc.dram_tensor("maskt", (S, QL), bf16, kind="ExternalInput")
    out_d = nc.dram_tensor("out", (QL, D), bf16, kind="ExternalOutput")
    scr_d = nc.dram_tensor("scr_sums", (1, QL), fp32)  # internal scratch

    with tile.TileContext(nc) as tc:
        with (
            tc.tile_pool(name="cons", bufs=1) as cons,
            tc.tile_pool(name="mp", bufs=1) as mp,
            tc.tile_pool(name="xqp", bufs=1) as xqp,
            tc.tile_pool(name="ktp", bufs=1) as ktp,
            tc.tile_pool(name="ptp", bufs=1) as ptp,
            tc.tile_pool(name="etp", bufs=1) as etp,
            tc.tile_pool(name="xkvp", bufs=1) as xkvp,
            tc.tile_pool(name="wvp", bufs=1) as wvp,
            tc.tile_pool(name="htp", bufs=1) as htp,
            tc.tile_pool(name="outp", bufs=2) as outp,
            tc.tile_pool(name="maskp", bufs=2) as maskp,
            tc.tile_pool(name="ps", bufs=ps_bufs,
                         space=bass.MemorySpace.PSUM) as psp,
            tc.tile_pool(name="pss", bufs=1, space=bass.MemorySpace.PSUM) as pssp,
        ):
            sy, sc, gp, ve = nc.sync, nc.scalar, nc.gpsimd, nc.vector

            # ---- SBUF tiles ----
            m_blks = [mp.tile([128, KA], bf16, tag=f"m{i}", name=f"m{i}")
                      for i in range(na)]
            xq_chs = [xqp.tile([128, na, AQCH[c]], bf16, tag=f"xq{c}",
                               name=f"xq{c}") for c in range(len(AQCH))]
            kt_sb = ktp.tile([128, NT_S, KA], bf16, tag="kt")
            pt_sb = ptp.tile([128, na, QL], bf16, tag="pt")
            et_sb = etp.tile([128, NT_S, QL], bf16, tag="et")
            xkv_sb = xkvp.tile([128, NV, NT_S, 128], bf16, tag="xkv")
            wv_sb = wvp.tile([128, NV, D], bf16, tag="wv")
            ht_sb = htp.tile([128, NV, QL], bf16, tag="ht")

            # ---- DMA issue schedule (priority order per engine) ----
            def ld_m(eng, i, lo=0, hi=None):
                hi = KA if hi is None else hi
                eng.dma_start(m_blks[i][:, lo:hi],
                              m_d.ap()[:, i * KA + lo:i * KA + hi])

            def ld_xq(eng, c, jlo, jhi):
                w = AQCH[c]
                base = na * AOFF[c]
                eng.dma_start(xq_chs[c][:, jlo:jhi, :],
                              xq_d.ap()[:, base + jlo * w:base + jhi * w])

            def ld_kt(eng, mlo, mhi):
                eng.dma_start(kt_sb[:, mlo:mhi, :],
                              kt_d.ap()[:, mlo * KA:mhi * KA])

            def ld_xkv(eng, j):
                eng.dma_start(xkv_sb[:, j, :, :],
                              xkv_d.ap()[:, j * S:(j + 1) * S])

            def ld_wv(eng, jlo, jhi):
                eng.dma_start(wv_sb[:, jlo:jhi, :],
                              wv_d.ap()[:, jlo * D:jhi * D])

            # startup-critical wave: first A group needs m0 + xq chunk 0;
            # m[i] is then consumed every ~1.7us. sync/scalar are HWDGE
            # queues (first payload ~8.4us from cold); gpsimd is SWDGE
            # (~2us slower to first byte) so it only gets pieces needed
            # a few us into stage A. Tiny dedicated leads land first.
            sy.dma_start(m_blks[0][:, 0:128], m_d.ap()[:, 0:128])
            sy.dma_start(xq_chs[0][:, 0:1, :], xq_d.ap()[:, 0:AQCH[0]])
            sc.dma_start(m_blks[0][:, 128:512], m_d.ap()[:, 128:512])
            gp.dma_start(xq_chs[0][:, 4:na, :],
                         xq_d.ap()[:, 4 * AQCH[0]:na * AQCH[0]])
            sy.dma_start(xq_chs[0][:, 2:4, :],
                         xq_d.ap()[:, 2 * AQCH[0]:4 * AQCH[0]])
            sc.dma_start(xq_chs[0][:, 1:2, :],
                         xq_d.ap()[:, AQCH[0]:2 * AQCH[0]])
            sy.dma_start(m_blks[0][:, 512:KA], m_d.ap()[:, 512:KA])
            ld_m(sy, 1)
            ld_m(gp, 2)
            ld_m(sy, 3); ld_m(gp, 4)
            ld_m(sy, 5); ld_m(gp, 6)
            ld_m(sy, 7)
            if na > 8:
                ld_m(gp, 8)

            # stage-B lhsT: 16 s-blocks streamed during B_n0 (from ~25us,
            # one per 1.7us); reused by B_n1.
            for m4 in range(0, NT_S, 4):
                ld_kt(gp, m4, m4 + 2)
                ld_kt(sy, m4 + 2, m4 + 4)

            # xq chunk 1 feeds A_c1 (~52us)
            ld_xq(sy, 1, 0, 4); ld_xq(gp, 1, 4, na)

            # stage-D operands (from ~94us) and stage-E weights (~150us)
            for j in range(NV):
                ld_xkv((sy, gp)[j % 2], j)
            ld_wv(gp, 0, 2); ld_wv(sy, 2, 4)
            ld_wv(gp, 4, 6); ld_wv(sy, 6, 8)

            # ---- constants (tiny, off the critical path) ----
            ones_col = cons.tile([128, 1], fp32, tag="ones_col")
            gp.memset(ones_col[:], 1.0)


            if with_bv:
                bv_sb = cons.tile([1, D], bf16, tag="bv")
                sy.dma_start(bv_sb[:], bv_d.ap()[:])

            # ---- stages A and B, interleaved by q-chunk ----
            # A_c0, B_n0, A_c1, B_n1: spreads the startup-critical DMA set
            # (m + xq chunk 0, then kt streamed during B_n0) across ~50us of
            # compute instead of demanding all of m in stage-A's first 7us.
            def a_chunk(c):
                w, off = AQCH[c], AOFF[c]
                for i in range(na):
                    ps = psp.tile([128, 512], fp32, tag="ps")
                    for j in range(na):
                        nc.tensor.matmul(
                            ps[:, :w], m_blks[i][:, j * 128:(j + 1) * 128],
                            xq_chs[c][:, j, :],
                            start=(j == 0), stop=(j == na - 1))
                    nc.scalar.activation(
                        pt_sb[:, i, off:off + w], ps[:, :w],
                        mybir.ActivationFunctionType.Copy)

            def b_chunk(n):
                for m in range(NT_S):
                    ps = psp.tile([128, 512], fp32, tag="ps")
                    for j in range(na):
                        nc.tensor.matmul(
                            ps[:], kt_sb[:, m, j * 128:(j + 1) * 128],
                            pt_sb[:, j, n * 512:(n + 1) * 512],
                            start=(j == 0), stop=(j == na - 1))
                    if with_mask:
                        mk = maskp.tile([128, 512], bf16, tag="mask")
                        sy.dma_start(
                            mk[:], maskt_d.ap()[m * 128:(m + 1) * 128,
                                                n * 512:(n + 1) * 512])
                        nc.vector.tensor_tensor(
                            ps[:], ps[:], mk[:], mybir.AluOpType.add)
                    nc.scalar.activation(
                        et_sb[:, m, n * 512:(n + 1) * 512], ps[:],
                        mybir.ActivationFunctionType.Exp, scale=SCALE)

            a_chunk(0)
            b_chunk(0)
            a_chunk(1)
            b_chunk(1)

            # softmax denominators on the DVE (PE has no slack; DVE does).
            sacc = cons.tile([128, QL], fp32, tag="sacc")
            nc.vector.tensor_tensor(
                sacc[:], et_sb[:, 0, :], et_sb[:, 1, :], mybir.AluOpType.add)
            for m in range(2, NT_S):
                nc.vector.tensor_tensor(
                    sacc[:], sacc[:], et_sb[:, m, :], mybir.AluOpType.add)

            sums_sb = cons.tile([1, QL], fp32, tag="sums")
            sums_t = cons.tile([128, 8], fp32, tag="sums_t")
            recip_sb = cons.tile([128, 8], fp32, tag="recip")
            if with_bv:
                sums_bf = cons.tile([1, QL], bf16, tag="sums_bf")

            # sums pieces woven between stage-D groups: the pss bank (bufs=1)
            # serializes on its ScalarE drain, and the [1,QL]->[128,8]
            # transpose runs as a DRAM round-trip DMA (off the PE; the old
            # 8 PE transposes cost ~2.4us of LDWEIGHTS), so a 3.4us D group
            # between pieces hides every handoff.
            def sums_piece(k):
                if k < NQ:
                    pss = pssp.tile([1, 512], fp32, tag="pss")
                    nc.tensor.matmul(
                        pss[:], ones_col[:], sacc[:, k * 512:(k + 1) * 512],
                        start=True, stop=True)
                    nc.scalar.activation(
                        sums_sb[:, k * 512:(k + 1) * 512], pss[:],
                        mybir.ActivationFunctionType.Copy)
                elif k == 2:
                    sy.dma_start(scr_d.ap()[:], sums_sb[:])
                elif k == 3:
                    # sums_t[p, c] = sums[c*128 + p]
                    sy.dma_start(
                        sums_t[:],
                        scr_d.ap().rearrange("a (c p) -> (a p) c", p=128))
                elif k == 4:
                    nc.vector.reciprocal(recip_sb[:], sums_t[:])
                    if with_bv:
                        # out accumulates UNNORMALIZED; bias enters as
                        # sums[q]*bv so the 1/sums scale leaves exactly +bv
                        nc.scalar.activation(
                            sums_bf[:], sums_sb[:],
                            mybir.ActivationFunctionType.Copy)

            # ---- stage D: HT = Xkv-tiles . eT ----
            for n in range(NQ):
                for j in range(NV):
                    ps = psp.tile([128, 512], fp32, tag="ps")
                    for m in range(NT_S):
                        nc.tensor.matmul(
                            ps[:], xkv_sb[:, j, m, :],
                            et_sb[:, m, n * 512:(n + 1) * 512],
                            start=(m == 0), stop=(m == NT_S - 1))
                    nc.scalar.activation(
                        ht_sb[:, j, n * 512:(n + 1) * 512], ps[:],
                        mybir.ActivationFunctionType.Copy)
                    if n == 1 and j >= 1:
                        sums_piece(j - 1)

            # ---- stage E: out = HT^T . Wv (+bv), normalized, bf16 out ----
            # p=0: matmuls first, then the sums block, then normalization.
            # The last p runs its final 512 cols as two 256-col groups with
            # act+store flushed per group, so only one short act+DMA chain
            # trails the last matmul.
            for p in range(8):
                out_sb = outp.tile([128, D], bf16, tag="outsb")
                widths = ([(0, 512), (512, 512)] if p < 7
                          else [(0, 512), (512, 256), (768, 256)])
                pending = []

                def flush(p=p, out_sb=out_sb, pending=pending):
                    for ps, lo, w in pending:
                        if with_bv:
                            nc.tensor.matmul(
                                ps[:, :w], sums_bf[:, p * 128:(p + 1) * 128],
                                bv_sb[:, lo:lo + w],
                                start=False, stop=True)
                        nc.scalar.activation(
                            out_sb[:, lo:lo + w], ps[:, :w],
                            mybir.ActivationFunctionType.Copy,
                            scale=recip_sb[:, p:p + 1])
                        # p=7 stores all go on sync: a store issued on the
                        # scalar stream would delay the final activations
                        eng = (nc.sync if p == 7 or (p + lo // 512) % 2 == 0
                               else nc.scalar)
                        eng.dma_start(
                            out_d.ap()[p * 128:(p + 1) * 128, lo:lo + w],
                            out_sb[:, lo:lo + w])
                    pending.clear()

                for lo, w in widths:
                    ps = psp.tile([128, 512], fp32, tag="ps")
                    for j in range(NV):
                        nc.tensor.matmul(
                            ps[:, :w], ht_sb[:, j, p * 128:(p + 1) * 128],
                            wv_sb[:, j, lo:lo + w],
                            start=(j == 0),
                            stop=(j == NV - 1 and not with_bv))
                    pending.append((ps, lo, w))
                    flush()

    nc.compile()
    return nc


def _get_nc(flags):
    if flags not in _cache:
        _cache[flags] = _build(*flags)
    return _cache[flags]


def _pack_blocks(X, nj, ni):
    # X [nj*128, ni*128] -> [128, ni*nj*128]: out[p, (i*nj+j)*128+c]
    # = X[j*128+p, i*128+c] (i-block-major SBUF tile layout, each i-block
    # one fully-contiguous DMA)
    return np.ascontiguousarray(
        X.reshape(nj, 128, ni, 128).transpose(1, 2, 0, 3)
        .reshape(128, ni * nj * 128))


def _pack_x(xt, widths, na):
    # xt [na*128, L] -> [128, sum_c na*w_c]: per col-chunk c,
    # block[p, j*w + t] = xt[j*128+p, off+t]
    blocks = []
    off = 0
    for w in widths:
        blk = xt[:, off:off + w].reshape(na, 128, w).transpose(1, 0, 2)
        blocks.append(blk.reshape(128, na * w))
        off += w
    return np.ascontiguousarray(np.concatenate(blocks, axis=1))


def _prep_in_maps(query_input, keyvalue_input, mask, Wq, bq, Wk, bk, Wv, bv):
    qi = np.asarray(query_input, np.float32)
    kv = np.asarray(keyvalue_input, np.float32)
    mask = np.asarray(mask, np.float32)
    Wq = np.asarray(Wq, np.float32)
    Wk = np.asarray(Wk, np.float32)
    Wvb = np.asarray(Wv, np.float32).astype(BF16)
    bq = np.asarray(bq, np.float32)
    bk = np.asarray(bk, np.float32)
    bv = np.asarray(bv, np.float32)

    with_mask = bool(np.any(mask != 0.0))
    with_bias = bool(np.any(bq != 0.0) or np.any(bk != 0.0))
    with_bv = bool(np.any(bv != 0.0))
    na = 9 if with_bias else 8
    KA = na * 128
    flags = (na, with_mask, with_bv)

    # M = Wq Wk^T (weights-only fold), augmented with q/k biases if nonzero
    M = np.zeros((KA, KA), np.float32)
    M[:D, :D] = Wq @ Wk.T
    if with_bias:
        M[:D, D] = Wq @ bk
        M[D, :D] = Wk @ bq
        M[D, D] = float(bq @ bk)
    M_p = _pack_blocks(M.astype(BF16), na, na)
    wv_p = np.ascontiguousarray(
        Wvb.reshape(8, 128, D).transpose(1, 0, 2).reshape(128, 8 * D))

    in_maps = []
    for c in range(N_CORES):
        b, h = c // 2, c % 2
        xq = qi[b, h * QL:(h + 1) * QL, :]          # [QL, D] fp32
        xkv = kv[b]                                  # [S, D] fp32
        xqt = np.zeros((KA, QL), np.float32)
        xqt[:D] = xq.T
        xkvt = np.zeros((KA, S), np.float32)
        xkvt[:D] = xkv.T
        if with_bias:
            xqt[D] = 1.0
            xkvt[D] = 1.0
        m_ = {
            "m": M_p,
            "xqt": _pack_x(xqt.astype(BF16), AQCH, na),
            "xkvt": _pack_blocks(xkvt.astype(BF16), na, NT_S),
            "xkv": _pack_blocks(xkv.astype(BF16), NT_S, 8),
            "wv": wv_p,
        }
        if with_bv:
            m_["bv"] = bv.astype(BF16).reshape(1, D)
        if with_mask:
            mt = mask[b, h * QL:(h + 1) * QL, :].T * np.float32(np.sqrt(D))
            m_["maskt"] = np.ascontiguousarray(mt.astype(BF16))
        in_maps.append(m_)
    return flags, in_maps


def _ensure_axon_hooks_stub():
    # bass_utils imports antenv.axon_hooks when tracing is requested; the
    # module is absent on some images, so register a no-op stub if needed.
    import sys, types
    try:
        import antenv.axon_hooks  # noqa: F401
    except ImportError:
        stub = types.ModuleType("antenv.axon_hooks")
        stub._hook = None
        stub.set_axon_ntff_profile_hook = (
            lambda h: setattr(stub, "_hook", h))
        stub.get_axon_ntff_profile_hook = lambda: stub._hook
        sys.modules["antenv.axon_hooks"] = stub
        try:
            import antenv
            antenv.axon_hooks = stub
        except ImportError:
            pass


def _run(inputs, trace=False, **kw):
    _ensure_axon_hooks_stub()
    from concourse import bass_utils
    ps_bufs = int(os.environ.get("KERNEL_PSBUFS", "6"))
    flags, in_maps = _prep_in_maps(**inputs)
    nc = _get_nc(flags + (ps_bufs,))
    res = bass_utils.run_bass_kernel_spmd(
        nc, in_maps, core_ids=list(range(N_CORES)), trace=trace, **kw)
    out = np.empty((B, S, D), np.float32)
    for c in range(N_CORES):
        b, h = c // 2, c % 2
        out[b, h * QL:(h + 1) * QL, :] = np.asarray(
            res.results[c]["out"], dtype=np.float32)
    return out, res


def kernel(**inputs) -> np.ndarray:
    out, _ = _run(inputs, trace=False)
    return out
